# revision 11
# baseline (speedup 1.0000x reference)
"""GAT (single-head, edge-featured) Trainium2 Bass kernel, 8-core SPMD.

Strategy (differs from the batch-sharding hint, chosen for DMA efficiency):
- Edges are sorted by destination on the host; core j owns dst nodes
  [1250j, 1250(j+1)) and every edge pointing into them. Softmax denominators
  and output rows are then core-local (no collectives).
- Node features live in a [N, 576]-f32 DRAM table per core:
  cols 0:512 = h[b,c] (all 8 graphs: one 2304B dma_gather descriptor fetches
  an edge's message for every graph), cols 512:520 = a_src[b],
  520:528 = a_dst[b]. Each core computes the full table (replicated work,
  avoids collectives).
- Per-edge work: dma_gather rows by src (messages+a_src) and 256B sub-rows by
  dst (a_dst); logits/exp on DVE+ACT; messages scaled by ex.
- Segment-sums (denominator and output rows) run on the PE via per-tile
  one-hot matmuls accumulated in PSUM windows of 128 dst nodes. Edge streams
  are padded per-window to a cross-core-uniform compile-time schedule.
  (dma_scatter_add loses updates on repeated indices, so no HBM scatter.)
- alpha = ex * r[dst] via a third small gather of r rows; host un-permutes.
"""
import os
import numpy as np

import concourse.bass as bass
import concourse.bacc as bacc
import concourse.mybir as mybir
import concourse.tile as tile
from concourse.bass_utils import run_bass_kernel_spmd

dt = mybir.dt
AF = mybir.ActivationFunctionType
ALU = mybir.AluOpType

B, N, E, F, C = 8, 10000, 320000, 128, 64
NC = 8                 # cores
NLOC = N // NC         # 1250 dst nodes per core
P = 128
NW = (NLOC + P - 1) // P   # 10 windows per core
ROWW = 576             # h_t row width (f32): 512 h | 8 a_src | 8 a_dst | 48 pad
CH = 1024              # edges per dma_gather call (HW ring limit ~1024-1536)
NT = (N + P - 1) // P  # 79 node tiles

_compiled = {}


def _build(EP, win_of, first_t, last_t):
    """Build the SPMD bass program. EP = padded edge count (same all cores),
    win_of[t] = window of tile t, first_t/last_t[w] = tile range of window w."""
    ES = EP // P          # edge slots
    NCH = EP // CH        # gather chunks
    nc = bacc.Bacc("TRN2")

    data_t = nc.declare_dram_parameter("data_t", [B, F, N], dt.float32, isOutput=False)
    W_in = nc.declare_dram_parameter("W", [F, C], dt.float32, isOutput=False)
    atts_r = nc.declare_dram_parameter("atts_r", [P, C], dt.float32, isOutput=False)
    attd_r = nc.declare_dram_parameter("attd_r", [P, C], dt.float32, isOutput=False)
    bias_r = nc.declare_dram_parameter("bias_r", [P, C], dt.float32, isOutput=False)
    we_row = nc.declare_dram_parameter("we_row", [1, C], dt.float32, isOutput=False)
    ae_row = nc.declare_dram_parameter("ae_row", [1, C], dt.float32, isOutput=False)
    ea_full = nc.declare_dram_parameter("ea_full", [P, E // P], dt.float32, isOutput=False)
    si_in = nc.declare_dram_parameter("si", [P, EP // 16], dt.int16, isOutput=False)
    dg_in = nc.declare_dram_parameter("dg", [P, EP // 16], dt.int16, isOutput=False)
    dl_in = nc.declare_dram_parameter("dl", [P, EP // 16], dt.int16, isOutput=False)
    ea_s_in = nc.declare_dram_parameter("ea_s", [P, ES], dt.float32, isOutput=False)
    loop_in = nc.declare_dram_parameter("loop_s", [P, ES], dt.float32, isOutput=False)
    mask_in = nc.declare_dram_parameter("mask_s", [P, ES], dt.float32, isOutput=False)
    drel_in = nc.declare_dram_parameter("drel_s", [P, ES], dt.float32, isOutput=False)
    iota_in = nc.declare_dram_parameter("iota_r", [P, P], dt.float32, isOutput=False)

    out_s = nc.declare_dram_parameter("out_s", [NW * P, B * C], dt.float32, isOutput=True)
    alpha_s = nc.declare_dram_parameter("alpha_s", [EP, B], dt.float32, isOutput=True)

    h_t = nc.dram_tensor("h_t", [N, ROWW], dt.float32)
    r_t = nc.dram_tensor("r_t", [NW * P, C], dt.float32)

    hwdge = None  # set inside

    with tile.TileContext(nc) as tc:
        with (
            tc.tile_pool(name="persist", bufs=1) as pp,
            tc.tile_pool(name="work", bufs=2) as wp,
            tc.tile_pool(name="oh", bufs=3) as ohp,
            tc.tile_pool(name="psum", bufs=2, space="PSUM") as psp,
        ):
            hwdge = [nc.sync, nc.scalar]

            # ---- load constants / index arrays ----
            si_t = pp.tile([P, EP // 16], dt.int16)
            dg_t = pp.tile([P, EP // 16], dt.int16)
            dl_t = pp.tile([P, EP // 16], dt.int16)
            ea_t = pp.tile([P, ES], dt.float32)
            loop_t = pp.tile([P, ES], dt.float32)
            mask_t = pp.tile([P, ES], dt.float32)
            drel_t = pp.tile([P, ES], dt.float32)
            iota_t = pp.tile([P, P], dt.float32)
            W_t = pp.tile([F, C], dt.float32)
            atts_t = pp.tile([P, C], dt.float32)
            attd_t = pp.tile([P, C], dt.float32)
            bias_t = pp.tile([P, C], dt.float32)
            we_t = pp.tile([1, C], dt.float32)
            ae_t = pp.tile([1, C], dt.float32)
            eaf_t = pp.tile([P, E // P], dt.float32)
            nc.sync.dma_start(out=si_t[:], in_=si_in[:])
            nc.sync.dma_start(out=dg_t[:], in_=dg_in[:])
            nc.sync.dma_start(out=dl_t[:], in_=dl_in[:])
            nc.scalar.dma_start(out=ea_t[:], in_=ea_s_in[:])
            nc.scalar.dma_start(out=loop_t[:], in_=loop_in[:])
            nc.scalar.dma_start(out=mask_t[:], in_=mask_in[:])
            nc.scalar.dma_start(out=drel_t[:], in_=drel_in[:])
            nc.scalar.dma_start(out=iota_t[:], in_=iota_in[:])
            nc.scalar.dma_start(out=W_t[:], in_=W_in[:])
            nc.scalar.dma_start(out=atts_t[:], in_=atts_r[:])
            nc.scalar.dma_start(out=attd_t[:], in_=attd_r[:])
            nc.scalar.dma_start(out=bias_t[:], in_=bias_r[:])
            nc.sync.dma_start(out=we_t[:], in_=we_row[:])
            nc.sync.dma_start(out=ae_t[:], in_=ae_row[:])
            nc.sync.dma_start(out=eaf_t[:], in_=ea_full[:])

            # ---- P0: scalars ----
            # mean(edge_attr): reduce free then partitions, scale by 1/E
            m1 = pp.tile([P, 1], dt.float32)
            m0 = pp.tile([1, 2], dt.float32)
            nc.vector.tensor_reduce(out=m1[:], in_=eaf_t[:], axis=mybir.AxisListType.X, op=ALU.add)
            nc.gpsimd.tensor_reduce(out=m0[:, 0:1], in_=m1[:], axis=mybir.AxisListType.C, op=ALU.add)
            nc.vector.tensor_scalar_mul(m0[:, 0:1], m0[:, 0:1], 1.0 / E)
            # s_edge = dot(W_edge, att_edge)
            se_v = pp.tile([1, C], dt.float32)
            nc.vector.tensor_tensor(out=se_v[:], in0=we_t[:], in1=ae_t[:], op=ALU.mult)
            nc.vector.tensor_reduce(out=m0[:, 1:2], in_=se_v[:], axis=mybir.AxisListType.X, op=ALU.add)
            # replicate (mean, s_edge) to all partitions via ones-matmul
            ones_t = pp.tile([P, P], dt.float32)
            z2 = pp.tile([P, 2], dt.float32)
            sc_t = pp.tile([P, 2], dt.float32)
            nc.vector.memset(ones_t[:], 1.0)
            nc.vector.memset(z2[:], 0.0)
            nc.vector.tensor_copy(z2[0:1, :], m0[:])
            psc = psp.tile([P, 2], dt.float32, tag="small")
            nc.tensor.matmul(psc[:], ones_t[:], z2[:], start=True, stop=True)
            nc.scalar.activation(out=sc_t[:], in_=psc[:], func=AF.Copy)
            # ea_eff = (ea + loop*mean) * s_edge
            eaef_t = pp.tile([P, ES], dt.float32)
            nc.vector.tensor_scalar(out=eaef_t[:], in0=loop_t[:], scalar1=sc_t[:, 0:1], scalar2=None, op0=ALU.mult)
            nc.vector.tensor_tensor(out=eaef_t[:], in0=eaef_t[:], in1=ea_t[:], op=ALU.add)
            nc.vector.tensor_scalar(out=eaef_t[:], in0=eaef_t[:], scalar1=sc_t[:, 1:2], scalar2=None, op0=ALU.mult)
            # w_src/w_dst = W @ att_{src,dst}
            wsd_t = pp.tile([F, 2], dt.float32)
            tmp_t = pp.tile([F, C], dt.float32)
            nc.vector.tensor_tensor(out=tmp_t[:], in0=W_t[:], in1=atts_t[:], op=ALU.mult)
            nc.vector.tensor_reduce(out=wsd_t[:, 0:1], in_=tmp_t[:], axis=mybir.AxisListType.X, op=ALU.add)
            nc.vector.tensor_tensor(out=tmp_t[:], in0=W_t[:], in1=attd_t[:], op=ALU.mult)
            nc.vector.tensor_reduce(out=wsd_t[:, 1:2], in_=tmp_t[:], axis=mybir.AxisListType.X, op=ALU.add)

            # ---- P1: node feature table h_t ----
            for nt in range(NT):
                n0 = nt * P
                rows = min(P, N - n0)
                ph = psp.tile([P, 512], dt.float32, tag="big")
                ps = psp.tile([P, 16], dt.float32, tag="small")
                for b in range(B):
                    dT = wp.tile([F, P], dt.float32, tag="dT")
                    hwdge[b % 2].dma_start(out=dT[:, 0:rows], in_=data_t[b, :, n0:n0 + rows])
                    nc.tensor.matmul(ph[0:rows, b * C:(b + 1) * C], dT[:, 0:rows], W_t[:],
                                     start=True, stop=True)
                    nc.tensor.matmul(ps[0:rows, 2 * b:2 * b + 2], dT[:, 0:rows], wsd_t[:],
                                     start=True, stop=True)
                stg = wp.tile([P, 512], dt.float32, tag="stg")
                stgs = wp.tile([P, 64], dt.float32, tag="stgs")
                nc.scalar.activation(out=stg[0:rows, :], in_=ph[0:rows, :], func=AF.Copy)
                nc.vector.memset(stgs[0:rows, 16:64], 0.0)
                nc.scalar.activation(out=stgs[0:rows, 0:16], in_=ps[0:rows, :], func=AF.Copy)
                nc.sync.dma_start(out=h_t[n0:n0 + rows, 0:512], in_=stg[0:rows, :])
                # h_t cols 512:528 keep psum layout: [a_src_b0, a_dst_b0, a_src_b1, ...]
                nc.scalar.dma_start(out=h_t[n0:n0 + rows, 512:576], in_=stgs[0:rows, :])

            # ---- P2: edge phase ----
            EX = pp.tile([P, ES, B], dt.float32)
            OUTA = pp.tile([P, NW, 512], dt.float32)
            DEN = pp.tile([P, NW, B], dt.float32)
            pm = None
            pe = None
            for c in range(NCH):
                G = wp.tile([P, CH // P, ROWW], dt.float32, tag="G")
                Dg = wp.tile([P, CH // P, C], dt.float32, tag="Dg")
                nc.gpsimd.dma_gather(
                    out_ap=G[:], in_ap=h_t[:], idxs_ap=si_t[:, 64 * c:64 * (c + 1)],
                    num_idxs=CH, num_idxs_reg=CH, elem_size=ROWW,
                )
                nc.gpsimd.dma_gather(
                    out_ap=Dg[:], in_ap=h_t[:, 512:576], idxs_ap=dg_t[:, 64 * c:64 * (c + 1)],
                    num_idxs=CH, num_idxs_reg=CH, elem_size=C, elem_step=ROWW,
                )
                L = wp.tile([P, CH // P, B], dt.float32, tag="L")
                exs = EX[:, (CH // P) * c:(CH // P) * (c + 1), :]
                nc.vector.tensor_tensor(
                    out=L[:].unsqueeze(3),
                    in0=G[:, :, 512:528].rearrange("p s (b t) -> p s b t", t=2)[:, :, :, 0:1],
                    in1=Dg[:, :, 0:16].rearrange("p s (b t) -> p s b t", t=2)[:, :, :, 1:2],
                    op=ALU.add)
                nc.vector.tensor_tensor(
                    out=L[:], in0=L[:],
                    in1=eaef_t[:, (CH // P) * c:(CH // P) * (c + 1)].unsqueeze(2).broadcast_to([P, CH // P, B]),
                    op=ALU.add)
                L2 = wp.tile([P, CH // P, B], dt.float32, tag="L2")
                nc.vector.tensor_scalar_mul(L2[:], L[:], 0.2)
                nc.vector.tensor_tensor(out=L[:], in0=L[:], in1=L2[:], op=ALU.max)
                nc.scalar.activation(out=exs, in_=L[:], func=AF.Exp)
                nc.vector.tensor_tensor(
                    out=exs, in0=exs,
                    in1=mask_t[:, (CH // P) * c:(CH // P) * (c + 1)].unsqueeze(2).broadcast_to([P, CH // P, B]),
                    op=ALU.mult)
                nc.vector.tensor_tensor(
                    out=G[:, :, 0:512].rearrange("p s (b f) -> p s b f", b=B),
                    in0=G[:, :, 0:512].rearrange("p s (b f) -> p s b f", b=B),
                    in1=exs.unsqueeze(3).broadcast_to([P, CH // P, B, C]),
                    op=ALU.mult)
                for ti in range(CH // P):
                    t = (CH // P) * c + ti
                    w = win_of[t]
                    oh = ohp.tile([P, P], dt.float32, tag="oh")
                    nc.vector.tensor_tensor(
                        out=oh[:], in0=drel_t[:, t:t + 1].broadcast_to([P, P]),
                        in1=iota_t[:], op=ALU.is_equal)
                    if t == first_t[w]:
                        pm = psp.tile([P, 512], dt.float32, tag="big")
                        pe = psp.tile([P, B], dt.float32, tag="small")
                    nc.tensor.matmul(pm[:], oh[:], G[:, ti, 0:512],
                                     start=(t == first_t[w]), stop=(t == last_t[w]))
                    nc.tensor.matmul(pe[:], oh[:], EX[:, t, :],
                                     start=(t == first_t[w]), stop=(t == last_t[w]))
                    if t == last_t[w]:
                        nc.scalar.activation(out=OUTA[:, w, :], in_=pm[:], func=AF.Copy)
                        nc.scalar.activation(out=DEN[:, w, :], in_=pe[:], func=AF.Copy)

            # ---- P3: r, alpha, out ----
            Rw = pp.tile([P, NW, C], dt.float32)
            nc.vector.memset(Rw[:], 0.0)
            nc.vector.tensor_scalar_add(DEN[:], DEN[:], 1e-16)
            nc.vector.reciprocal(Rw[:, :, 0:B], DEN[:])
            R = Rw[:, :, 0:B]
            nc.sync.dma_start(
                out=r_t[:].rearrange("(w p) c -> p w c", p=P), in_=Rw[:])
            for c in range(NCH):
                Rg = wp.tile([P, CH // P, C], dt.float32, tag="Rg")
                nc.gpsimd.dma_gather(
                    out_ap=Rg[:], in_ap=r_t[:], idxs_ap=dl_t[:, 64 * c:64 * (c + 1)],
                    num_idxs=CH, num_idxs_reg=CH, elem_size=C,
                )
                A = wp.tile([P, CH // P, B], dt.float32, tag="A")
                nc.vector.tensor_tensor(
                    out=A[:], in0=EX[:, (CH // P) * c:(CH // P) * (c + 1), :],
                    in1=Rg[:, :, 0:B], op=ALU.mult)
                nc.scalar.dma_start(
                    out=alpha_s[:].rearrange("(s p) b -> p s b", p=P)[:, (CH // P) * c:(CH // P) * (c + 1), :],
                    in_=A[:])
            # out = OUTA * r + bias
            nc.vector.tensor_tensor(
                out=OUTA[:].rearrange("p w (b f) -> p w b f", b=B),
                in0=OUTA[:].rearrange("p w (b f) -> p w b f", b=B),
                in1=R.unsqueeze(3).broadcast_to([P, NW, B, C]),
                op=ALU.mult)
            nc.vector.tensor_tensor(
                out=OUTA[:].rearrange("p w (b f) -> p w b f", b=B),
                in0=OUTA[:].rearrange("p w (b f) -> p w b f", b=B),
                in1=bias_t[:].unsqueeze(1).unsqueeze(1).broadcast_to([P, NW, B, C]),
                op=ALU.add)
            nc.sync.dma_start(
                out=out_s[:].rearrange("(w p) c -> p w c", p=P), in_=OUTA[:])
    nc.compile()
    return nc


def _wrap16(a):
    w = np.ascontiguousarray(a.reshape(-1, 16).T.astype(np.int16))
    return np.tile(w, (NC, 1))


def _slotf(a):
    # edge i -> [i % 128, i // 128], f32
    return np.ascontiguousarray(a.reshape(-1, P).T.astype(np.float32))


def kernel(data, edge_index, edge_attr, W, att_src, att_dst, W_edge, att_edge, bias):
    data = np.asarray(data, dtype=np.float32)
    edge_index = np.asarray(edge_index)
    edge_attr = np.asarray(edge_attr, dtype=np.float32)
    W = np.asarray(W, dtype=np.float32)
    att_src = np.asarray(att_src, dtype=np.float32)
    att_dst = np.asarray(att_dst, dtype=np.float32)
    W_edge = np.asarray(W_edge, dtype=np.float32)
    att_edge = np.asarray(att_edge, dtype=np.float32)
    bias = np.asarray(bias, dtype=np.float32)

    # ---- host preprocessing: sort by dst, carve per-core windows ----
    src_a = np.concatenate([edge_index[0], np.arange(N)]).astype(np.int64)
    dst_a = np.concatenate([edge_index[1], np.arange(N)]).astype(np.int64)
    ea_a = np.concatenate([edge_attr[:, 0], np.zeros(N, np.float32)])
    lf_a = np.concatenate([np.zeros(E, np.float32), np.ones(N, np.float32)])
    EA = E + N

    perm = np.argsort(dst_a, kind="stable")
    ds, ss = dst_a[perm], src_a[perm]
    eas, lfs = ea_a[perm], lf_a[perm]

    bnds = np.array([j * NLOC + min(w * P, NLOC) for j in range(NC) for w in range(NW)] + [N])
    lo = np.searchsorted(ds, bnds)
    cnt = np.diff(lo).reshape(NC, NW)
    Tw = -(-cnt.max(axis=0) // P)          # tiles per window
    tiles = int(Tw.sum())
    tiles = -(-tiles // (CH // P)) * (CH // P)   # round to whole chunks
    extra = tiles - int(Tw.sum())
    Tw_list = list(Tw)
    Tw_list[-1] += extra                   # null tiles appended to last window
    EP = tiles * P

    win_of, first_t, last_t = [], [0] * NW, [0] * NW
    t0 = 0
    for w in range(NW):
        first_t[w] = t0
        last_t[w] = t0 + Tw_list[w] - 1
        win_of += [w] * Tw_list[w]
        t0 += Tw_list[w]

    # per-core padded edge streams
    in_maps = []
    data_tr = np.ascontiguousarray(np.transpose(data, (0, 2, 1)))
    ea_w = np.ascontiguousarray(edge_attr[:, 0].reshape(-1, P).T)
    iota_r = np.tile(np.arange(P, dtype=np.float32), (P, 1))
    shared = dict(
        data_t=data_tr, W=W,
        atts_r=np.tile(att_src, (P, 1)).astype(np.float32),
        attd_r=np.tile(att_dst, (P, 1)).astype(np.float32),
        bias_r=np.tile(bias, (P, 1)).astype(np.float32),
        we_row=W_edge.reshape(1, C), ae_row=att_edge.reshape(1, C),
        ea_full=ea_w, iota_r=iota_r,
    )
    meta = []
    for j in range(NC):
        si = np.zeros(EP, np.int64)
        dg = np.zeros(EP, np.int64)
        dl = np.zeros(EP, np.int64)
        eaj = np.zeros(EP, np.float32)
        lfj = np.zeros(EP, np.float32)
        mkj = np.zeros(EP, np.float32)
        drj = np.full(EP, -1.0, np.float32)
        seg = []
        for w in range(NW):
            s0 = first_t[w] * P
            k0, k1 = lo[j * NW + w], lo[j * NW + w] + cnt[j, w]
            n = cnt[j, w]
            si[s0:s0 + n] = ss[k0:k1]
            dg[s0:s0 + n] = ds[k0:k1]
            dl[s0:s0 + n] = ds[k0:k1] - j * NLOC
            eaj[s0:s0 + n] = eas[k0:k1]
            lfj[s0:s0 + n] = lfs[k0:k1]
            mkj[s0:s0 + n] = 1.0
            drj[s0:s0 + n] = (ds[k0:k1] - j * NLOC - w * P).astype(np.float32)
            seg.append((s0, int(n), int(k0), int(k1)))
        meta.append(seg)
        m = dict(shared)
        m.update(
            si=_wrap16(si), dg=_wrap16(dg), dl=_wrap16(dl),
            ea_s=_slotf(eaj), loop_s=_slotf(lfj), mask_s=_slotf(mkj), drel_s=_slotf(drj),
        )
        in_maps.append(m)

    key = (EP, tuple(win_of))
    if key not in _compiled:
        _compiled[key] = _build(EP, win_of, first_t, last_t)
    nc = _compiled[key]

    trace = bool(os.environ.get("GAT_TRACE"))
    res = run_bass_kernel_spmd(nc, in_maps, list(range(NC)), trace=trace,
                               tmpdir=os.environ.get("GAT_TRACE_DIR") or None)
    if trace and res.exec_time_ns is not None:
        print(f"HW exec time: {res.exec_time_ns} ns")

    out = np.empty((B, N, C), np.float32)
    alpha = np.empty((B, EA), np.float32)
    for j in range(NC):
        rj = res.results[j]
        o = np.asarray(rj["out_s"]).reshape(NW * P, B, C)[:NLOC]
        out[:, j * NLOC:(j + 1) * NLOC, :] = np.transpose(o, (1, 0, 2))
        aj = np.asarray(rj["alpha_s"])
        for (s0, n, k0, k1) in meta[j]:
            alpha[:, perm[k0:k1]] = aj[s0:s0 + n, :].T
    return out, alpha


# revision 12
# speedup vs baseline: 1.0825x; 1.0825x over previous
"""GAT (single-head, edge-featured) Trainium2 Bass kernel, 8-core SPMD.

Strategy (differs from the batch-sharding hint, chosen for DMA efficiency):
- Edges are sorted by destination on the host; core j owns dst nodes
  [1250j, 1250(j+1)) and every edge pointing into them. Softmax denominators
  and output rows are then core-local (no collectives).
- Node features live in a [N, 576]-f32 DRAM table per core:
  cols 0:512 = h[b,c] (all 8 graphs: one 2304B dma_gather descriptor fetches
  an edge's message for every graph), cols 512:520 = a_src[b],
  520:528 = a_dst[b]. Each core computes the full table (replicated work,
  avoids collectives).
- Per-edge work: dma_gather rows by src (messages+a_src) and 256B sub-rows by
  dst (a_dst); logits/exp on DVE+ACT; messages scaled by ex.
- Segment-sums (denominator and output rows) run on the PE via per-tile
  one-hot matmuls accumulated in PSUM windows of 128 dst nodes. Edge streams
  are padded per-window to a cross-core-uniform compile-time schedule.
  (dma_scatter_add loses updates on repeated indices, so no HBM scatter.)
- alpha = ex * r[dst] via a third small gather of r rows; host un-permutes.
"""
import os
import numpy as np

import concourse.bass as bass
import concourse.bacc as bacc
import concourse.mybir as mybir
import concourse.tile as tile
from concourse.bass_utils import run_bass_kernel_spmd

dt = mybir.dt
AF = mybir.ActivationFunctionType
ALU = mybir.AluOpType

B, N, E, F, C = 8, 10000, 320000, 128, 64
NC = 8                 # cores
NLOC = N // NC         # 1250 dst nodes per core
P = 128
NW = (NLOC + P - 1) // P   # 10 windows per core
ROWW = 576             # h_t row width (f32): 512 h | 8 a_src | 8 a_dst | 48 pad
CH = 1024              # edges per dma_gather call (HW ring limit ~1024-1536)
NT = (N + P - 1) // P  # 79 node tiles

_compiled = {}


def _build(EP, win_of, first_t, last_t):
    """Build the SPMD bass program. EP = padded edge count (same all cores),
    win_of[t] = window of tile t, first_t/last_t[w] = tile range of window w."""
    ES = EP // P          # edge slots
    NCH = EP // CH        # gather chunks
    nc = bacc.Bacc("TRN2")

    data_t = nc.declare_dram_parameter("data_t", [B, F, N], dt.float32, isOutput=False)
    W_in = nc.declare_dram_parameter("W", [F, C], dt.float32, isOutput=False)
    atts_r = nc.declare_dram_parameter("atts_r", [P, C], dt.float32, isOutput=False)
    attd_r = nc.declare_dram_parameter("attd_r", [P, C], dt.float32, isOutput=False)
    bias_r = nc.declare_dram_parameter("bias_r", [P, C], dt.float32, isOutput=False)
    we_row = nc.declare_dram_parameter("we_row", [1, C], dt.float32, isOutput=False)
    ae_row = nc.declare_dram_parameter("ae_row", [1, C], dt.float32, isOutput=False)
    ea_full = nc.declare_dram_parameter("ea_full", [P, E // P], dt.float32, isOutput=False)
    si_in = nc.declare_dram_parameter("si", [P, EP // 16], dt.int16, isOutput=False)
    dg_in = nc.declare_dram_parameter("dg", [P, EP // 16], dt.int16, isOutput=False)
    dl_in = nc.declare_dram_parameter("dl", [P, EP // 16], dt.int16, isOutput=False)
    ea_s_in = nc.declare_dram_parameter("ea_s", [P, ES], dt.float32, isOutput=False)
    loop_in = nc.declare_dram_parameter("loop_s", [P, ES], dt.float32, isOutput=False)
    mask_in = nc.declare_dram_parameter("mask_s", [P, ES], dt.float32, isOutput=False)
    drel_in = nc.declare_dram_parameter("drel_s", [P, ES], dt.float32, isOutput=False)
    iota_in = nc.declare_dram_parameter("iota_r", [P, P], dt.float32, isOutput=False)
    iotac_in = nc.declare_dram_parameter("iota_c", [P, P], dt.float32, isOutput=False)
    drelT_in = nc.declare_dram_parameter("drelT", [1, EP], dt.float32, isOutput=False)

    out_s = nc.declare_dram_parameter("out_s", [NW * P, B * C], dt.float32, isOutput=True)
    alpha_s = nc.declare_dram_parameter("alpha_s", [EP, B], dt.float32, isOutput=True)

    h_t = nc.dram_tensor("h_t", [N, ROWW], dt.float32)

    hwdge = None  # set inside

    with tile.TileContext(nc) as tc:
        with (
            tc.tile_pool(name="persist", bufs=1) as pp,
            tc.tile_pool(name="work", bufs=2) as wp,
            tc.tile_pool(name="oh", bufs=3) as ohp,
            tc.tile_pool(name="psum", bufs=2, space="PSUM") as psp,
        ):
            hwdge = [nc.sync, nc.scalar]

            # ---- load constants / index arrays ----
            si_t = pp.tile([P, EP // 16], dt.int16)
            dg_t = pp.tile([P, EP // 16], dt.int16)
            ea_t = pp.tile([P, ES], dt.float32)
            loop_t = pp.tile([P, ES], dt.float32)
            mask_t = pp.tile([P, ES], dt.float32)
            drel_t = pp.tile([P, ES], dt.float32)
            iota_t = pp.tile([P, P], dt.float32)
            iotac_t = pp.tile([P, P], dt.float32)
            W_t = pp.tile([F, C], dt.float32)
            atts_t = pp.tile([P, C], dt.float32)
            attd_t = pp.tile([P, C], dt.float32)
            bias_t = pp.tile([P, C], dt.float32)
            we_t = pp.tile([1, C], dt.float32)
            ae_t = pp.tile([1, C], dt.float32)
            eaf_t = pp.tile([P, E // P], dt.float32)
            nc.sync.dma_start(out=si_t[:], in_=si_in[:])
            nc.sync.dma_start(out=dg_t[:], in_=dg_in[:])
            nc.scalar.dma_start(out=ea_t[:], in_=ea_s_in[:])
            nc.scalar.dma_start(out=loop_t[:], in_=loop_in[:])
            nc.scalar.dma_start(out=mask_t[:], in_=mask_in[:])
            nc.scalar.dma_start(out=drel_t[:], in_=drel_in[:])
            nc.scalar.dma_start(out=iota_t[:], in_=iota_in[:])
            nc.scalar.dma_start(out=iotac_t[:], in_=iotac_in[:])
            nc.scalar.dma_start(out=W_t[:], in_=W_in[:])
            nc.scalar.dma_start(out=atts_t[:], in_=atts_r[:])
            nc.scalar.dma_start(out=attd_t[:], in_=attd_r[:])
            nc.scalar.dma_start(out=bias_t[:], in_=bias_r[:])
            nc.sync.dma_start(out=we_t[:], in_=we_row[:])
            nc.sync.dma_start(out=ae_t[:], in_=ae_row[:])
            nc.sync.dma_start(out=eaf_t[:], in_=ea_full[:])

            # ---- P0: scalars ----
            # mean(edge_attr): reduce free then partitions, scale by 1/E
            m1 = pp.tile([P, 1], dt.float32)
            m0 = pp.tile([1, 2], dt.float32)
            nc.vector.tensor_reduce(out=m1[:], in_=eaf_t[:], axis=mybir.AxisListType.X, op=ALU.add)
            nc.gpsimd.tensor_reduce(out=m0[:, 0:1], in_=m1[:], axis=mybir.AxisListType.C, op=ALU.add)
            nc.vector.tensor_scalar_mul(m0[:, 0:1], m0[:, 0:1], 1.0 / E)
            # s_edge = dot(W_edge, att_edge)
            se_v = pp.tile([1, C], dt.float32)
            nc.vector.tensor_tensor(out=se_v[:], in0=we_t[:], in1=ae_t[:], op=ALU.mult)
            nc.vector.tensor_reduce(out=m0[:, 1:2], in_=se_v[:], axis=mybir.AxisListType.X, op=ALU.add)
            # replicate (mean, s_edge) to all partitions via ones-matmul
            ones_t = pp.tile([P, P], dt.float32)
            z2 = pp.tile([P, 2], dt.float32)
            sc_t = pp.tile([P, 2], dt.float32)
            nc.vector.memset(ones_t[:], 1.0)
            nc.vector.memset(z2[:], 0.0)
            nc.vector.tensor_copy(z2[0:1, :], m0[:])
            psc = psp.tile([P, 2], dt.float32, tag="small")
            nc.tensor.matmul(psc[:], ones_t[:], z2[:], start=True, stop=True)
            nc.scalar.activation(out=sc_t[:], in_=psc[:], func=AF.Copy)
            # ea_eff = (ea + loop*mean) * s_edge
            eaef_t = pp.tile([P, ES], dt.float32)
            nc.vector.tensor_scalar(out=eaef_t[:], in0=loop_t[:], scalar1=sc_t[:, 0:1], scalar2=None, op0=ALU.mult)
            nc.vector.tensor_tensor(out=eaef_t[:], in0=eaef_t[:], in1=ea_t[:], op=ALU.add)
            nc.vector.tensor_scalar(out=eaef_t[:], in0=eaef_t[:], scalar1=sc_t[:, 1:2], scalar2=None, op0=ALU.mult)
            # w_src/w_dst = W @ att_{src,dst}
            wsd_t = pp.tile([F, 2], dt.float32)
            tmp_t = pp.tile([F, C], dt.float32)
            nc.vector.tensor_tensor(out=tmp_t[:], in0=W_t[:], in1=atts_t[:], op=ALU.mult)
            nc.vector.tensor_reduce(out=wsd_t[:, 0:1], in_=tmp_t[:], axis=mybir.AxisListType.X, op=ALU.add)
            nc.vector.tensor_tensor(out=tmp_t[:], in0=W_t[:], in1=attd_t[:], op=ALU.mult)
            nc.vector.tensor_reduce(out=wsd_t[:, 1:2], in_=tmp_t[:], axis=mybir.AxisListType.X, op=ALU.add)

            # ---- P1: node feature table h_t ----
            for nt in range(NT):
                n0 = nt * P
                rows = min(P, N - n0)
                ph = psp.tile([P, 512], dt.float32, tag="big")
                ps = psp.tile([P, 16], dt.float32, tag="small")
                for b in range(B):
                    dT = wp.tile([F, P], dt.float32, tag="dT")
                    hwdge[b % 2].dma_start(out=dT[:, 0:rows], in_=data_t[b, :, n0:n0 + rows])
                    nc.tensor.matmul(ph[0:rows, b * C:(b + 1) * C], dT[:, 0:rows], W_t[:],
                                     start=True, stop=True)
                    nc.tensor.matmul(ps[0:rows, 2 * b:2 * b + 2], dT[:, 0:rows], wsd_t[:],
                                     start=True, stop=True)
                stg = wp.tile([P, 512], dt.float32, tag="stg")
                stgs = wp.tile([P, 64], dt.float32, tag="stgs")
                nc.scalar.activation(out=stg[0:rows, :], in_=ph[0:rows, :], func=AF.Copy)
                nc.vector.memset(stgs[0:rows, 16:64], 0.0)
                nc.scalar.activation(out=stgs[0:rows, 0:16], in_=ps[0:rows, :], func=AF.Copy)
                nc.sync.dma_start(out=h_t[n0:n0 + rows, 0:512], in_=stg[0:rows, :])
                # h_t cols 512:528 keep psum layout: [a_src_b0, a_dst_b0, a_src_b1, ...]
                nc.scalar.dma_start(out=h_t[n0:n0 + rows, 512:576], in_=stgs[0:rows, :])

            # ---- P2: edge phase ----
            EX = pp.tile([P, ES, B], dt.float32)
            OUTA = pp.tile([P, NW, 512], dt.float32)
            DEN = pp.tile([P, NW, B], dt.float32)
            pm = None
            pe = None
            for c in range(NCH):
                G = wp.tile([P, CH // P, ROWW], dt.float32, tag="G")
                Dg = wp.tile([P, CH // P, C], dt.float32, tag="Dg")
                nc.gpsimd.dma_gather(
                    out_ap=G[:], in_ap=h_t[:], idxs_ap=si_t[:, 64 * c:64 * (c + 1)],
                    num_idxs=CH, num_idxs_reg=CH, elem_size=ROWW,
                )
                nc.gpsimd.dma_gather(
                    out_ap=Dg[:], in_ap=h_t[:, 512:576], idxs_ap=dg_t[:, 64 * c:64 * (c + 1)],
                    num_idxs=CH, num_idxs_reg=CH, elem_size=C, elem_step=ROWW,
                )
                L = wp.tile([P, CH // P, B], dt.float32, tag="L")
                exs = EX[:, (CH // P) * c:(CH // P) * (c + 1), :]
                nc.vector.tensor_tensor(
                    out=L[:].unsqueeze(3),
                    in0=G[:, :, 512:528].rearrange("p s (b t) -> p s b t", t=2)[:, :, :, 0:1],
                    in1=Dg[:, :, 0:16].rearrange("p s (b t) -> p s b t", t=2)[:, :, :, 1:2],
                    op=ALU.add)
                nc.vector.tensor_tensor(
                    out=L[:], in0=L[:],
                    in1=eaef_t[:, (CH // P) * c:(CH // P) * (c + 1)].unsqueeze(2).broadcast_to([P, CH // P, B]),
                    op=ALU.add)
                L2 = wp.tile([P, CH // P, B], dt.float32, tag="L2")
                nc.vector.tensor_scalar_mul(L2[:], L[:], 0.2)
                nc.vector.tensor_tensor(out=L[:], in0=L[:], in1=L2[:], op=ALU.max)
                nc.scalar.activation(out=exs, in_=L[:], func=AF.Exp)
                nc.vector.tensor_tensor(
                    out=exs, in0=exs,
                    in1=mask_t[:, (CH // P) * c:(CH // P) * (c + 1)].unsqueeze(2).broadcast_to([P, CH // P, B]),
                    op=ALU.mult)
                nc.vector.tensor_tensor(
                    out=G[:, :, 0:512].rearrange("p s (b f) -> p s b f", b=B),
                    in0=G[:, :, 0:512].rearrange("p s (b f) -> p s b f", b=B),
                    in1=exs.unsqueeze(3).broadcast_to([P, CH // P, B, C]),
                    op=ALU.mult)
                for ti in range(CH // P):
                    t = (CH // P) * c + ti
                    w = win_of[t]
                    oh = ohp.tile([P, P], dt.float32, tag="oh")
                    nc.vector.tensor_tensor(
                        out=oh[:], in0=drel_t[:, t:t + 1].broadcast_to([P, P]),
                        in1=iota_t[:], op=ALU.is_equal)
                    if t == first_t[w]:
                        pm = psp.tile([P, 512], dt.float32, tag="big")
                        pe = psp.tile([P, B], dt.float32, tag="small")
                    nc.tensor.matmul(pm[:], oh[:], G[:, ti, 0:512],
                                     start=(t == first_t[w]), stop=(t == last_t[w]))
                    nc.tensor.matmul(pe[:], oh[:], EX[:, t, :],
                                     start=(t == first_t[w]), stop=(t == last_t[w]))
                    if t == last_t[w]:
                        nc.scalar.activation(out=OUTA[:, w, :], in_=pm[:], func=AF.Copy)
                        nc.scalar.activation(out=DEN[:, w, :], in_=pe[:], func=AF.Copy)

            # ---- P3: r, alpha, out ----
            Rw = pp.tile([P, NW, B], dt.float32)
            nc.vector.tensor_scalar_add(DEN[:], DEN[:], 1e-16)
            nc.vector.reciprocal(Rw[:], DEN[:])
            R = Rw[:]
            for c in range(NCH):
                dbt = wp.tile([P, CH], dt.float32, tag="dbt")
                nc.sync.dma_start(out=dbt[:], in_=drelT_in[0:1, CH * c:CH * (c + 1)].broadcast_to([P, CH]))
                A = wp.tile([P, CH // P, B], dt.float32, tag="A")
                for ti in range(CH // P):
                    t = (CH // P) * c + ti
                    w = win_of[t]
                    ohT = ohp.tile([P, P], dt.float32, tag="ohT")
                    nc.vector.tensor_tensor(
                        out=ohT[:], in0=iotac_t[:], in1=dbt[:, P * ti:P * (ti + 1)],
                        op=ALU.is_equal)
                    pa = psp.tile([P, 16], dt.float32, tag="small")
                    nc.tensor.matmul(pa[:, 0:B], ohT[:], Rw[:, w, :], start=True, stop=True)
                    nc.vector.tensor_tensor(
                        out=A[:, ti, :], in0=EX[:, t, :], in1=pa[:, 0:B], op=ALU.mult)
                nc.scalar.dma_start(
                    out=alpha_s[:].rearrange("(s p) b -> p s b", p=P)[:, (CH // P) * c:(CH // P) * (c + 1), :],
                    in_=A[:])
            # out = OUTA * r + bias
            nc.vector.tensor_tensor(
                out=OUTA[:].rearrange("p w (b f) -> p w b f", b=B),
                in0=OUTA[:].rearrange("p w (b f) -> p w b f", b=B),
                in1=Rw[:].unsqueeze(3).broadcast_to([P, NW, B, C]),
                op=ALU.mult)
            nc.vector.tensor_tensor(
                out=OUTA[:].rearrange("p w (b f) -> p w b f", b=B),
                in0=OUTA[:].rearrange("p w (b f) -> p w b f", b=B),
                in1=bias_t[:].unsqueeze(1).unsqueeze(1).broadcast_to([P, NW, B, C]),
                op=ALU.add)
            nc.sync.dma_start(
                out=out_s[:].rearrange("(w p) c -> p w c", p=P), in_=OUTA[:])
    nc.compile()
    return nc


def _wrap16(a):
    w = np.ascontiguousarray(a.reshape(-1, 16).T.astype(np.int16))
    return np.tile(w, (NC, 1))


def _slotf(a):
    # edge i -> [i % 128, i // 128], f32
    return np.ascontiguousarray(a.reshape(-1, P).T.astype(np.float32))


def kernel(data, edge_index, edge_attr, W, att_src, att_dst, W_edge, att_edge, bias):
    data = np.asarray(data, dtype=np.float32)
    edge_index = np.asarray(edge_index)
    edge_attr = np.asarray(edge_attr, dtype=np.float32)
    W = np.asarray(W, dtype=np.float32)
    att_src = np.asarray(att_src, dtype=np.float32)
    att_dst = np.asarray(att_dst, dtype=np.float32)
    W_edge = np.asarray(W_edge, dtype=np.float32)
    att_edge = np.asarray(att_edge, dtype=np.float32)
    bias = np.asarray(bias, dtype=np.float32)

    # ---- host preprocessing: sort by dst, carve per-core windows ----
    src_a = np.concatenate([edge_index[0], np.arange(N)]).astype(np.int64)
    dst_a = np.concatenate([edge_index[1], np.arange(N)]).astype(np.int64)
    ea_a = np.concatenate([edge_attr[:, 0], np.zeros(N, np.float32)])
    lf_a = np.concatenate([np.zeros(E, np.float32), np.ones(N, np.float32)])
    EA = E + N

    perm = np.argsort(dst_a, kind="stable")
    ds, ss = dst_a[perm], src_a[perm]
    eas, lfs = ea_a[perm], lf_a[perm]

    bnds = np.array([j * NLOC + min(w * P, NLOC) for j in range(NC) for w in range(NW)] + [N])
    lo = np.searchsorted(ds, bnds)
    cnt = np.diff(lo).reshape(NC, NW)
    Tw = -(-cnt.max(axis=0) // P)          # tiles per window
    tiles = int(Tw.sum())
    tiles = -(-tiles // (CH // P)) * (CH // P)   # round to whole chunks
    extra = tiles - int(Tw.sum())
    Tw_list = list(Tw)
    Tw_list[-1] += extra                   # null tiles appended to last window
    EP = tiles * P

    win_of, first_t, last_t = [], [0] * NW, [0] * NW
    t0 = 0
    for w in range(NW):
        first_t[w] = t0
        last_t[w] = t0 + Tw_list[w] - 1
        win_of += [w] * Tw_list[w]
        t0 += Tw_list[w]

    # per-core padded edge streams
    in_maps = []
    data_tr = np.ascontiguousarray(np.transpose(data, (0, 2, 1)))
    ea_w = np.ascontiguousarray(edge_attr[:, 0].reshape(-1, P).T)
    iota_r = np.tile(np.arange(P, dtype=np.float32), (P, 1))
    iota_c = np.ascontiguousarray(iota_r.T)
    shared = dict(
        data_t=data_tr, W=W,
        atts_r=np.tile(att_src, (P, 1)).astype(np.float32),
        attd_r=np.tile(att_dst, (P, 1)).astype(np.float32),
        bias_r=np.tile(bias, (P, 1)).astype(np.float32),
        we_row=W_edge.reshape(1, C), ae_row=att_edge.reshape(1, C),
        ea_full=ea_w, iota_r=iota_r, iota_c=iota_c,
    )
    meta = []
    for j in range(NC):
        si = np.zeros(EP, np.int64)
        dg = np.zeros(EP, np.int64)
        dl = np.zeros(EP, np.int64)
        eaj = np.zeros(EP, np.float32)
        lfj = np.zeros(EP, np.float32)
        mkj = np.zeros(EP, np.float32)
        drj = np.full(EP, -1.0, np.float32)
        seg = []
        for w in range(NW):
            s0 = first_t[w] * P
            k0, k1 = lo[j * NW + w], lo[j * NW + w] + cnt[j, w]
            n = cnt[j, w]
            si[s0:s0 + n] = ss[k0:k1]
            dg[s0:s0 + n] = ds[k0:k1]
            dl[s0:s0 + n] = ds[k0:k1] - j * NLOC
            eaj[s0:s0 + n] = eas[k0:k1]
            lfj[s0:s0 + n] = lfs[k0:k1]
            mkj[s0:s0 + n] = 1.0
            drj[s0:s0 + n] = (ds[k0:k1] - j * NLOC - w * P).astype(np.float32)
            seg.append((s0, int(n), int(k0), int(k1)))
        meta.append(seg)
        m = dict(shared)
        m.update(
            si=_wrap16(si), dg=_wrap16(dg), dl=_wrap16(dl), drelT=drj.reshape(1, -1),
            ea_s=_slotf(eaj), loop_s=_slotf(lfj), mask_s=_slotf(mkj), drel_s=_slotf(drj),
        )
        in_maps.append(m)

    key = (EP, tuple(win_of))
    if key not in _compiled:
        _compiled[key] = _build(EP, win_of, first_t, last_t)
    nc = _compiled[key]

    trace = bool(os.environ.get("GAT_TRACE"))
    res = run_bass_kernel_spmd(nc, in_maps, list(range(NC)), trace=trace,
                               tmpdir=os.environ.get("GAT_TRACE_DIR") or None)
    if trace and res.exec_time_ns is not None:
        print(f"HW exec time: {res.exec_time_ns} ns")

    out = np.empty((B, N, C), np.float32)
    alpha = np.empty((B, EA), np.float32)
    for j in range(NC):
        rj = res.results[j]
        o = np.asarray(rj["out_s"]).reshape(NW * P, B, C)[:NLOC]
        out[:, j * NLOC:(j + 1) * NLOC, :] = np.transpose(o, (1, 0, 2))
        aj = np.asarray(rj["alpha_s"])
        for (s0, n, k0, k1) in meta[j]:
            alpha[:, perm[k0:k1]] = aj[s0:s0 + n, :].T
    return out, alpha
